# revision 21
# baseline (speedup 1.0000x reference)
"""Trainium2 Bass kernel for nn_Neighbor_Mean (gnn message passing).

Math: out[b,s,:] = mean_n( mask[b,s,n] * (T_b[idx[b,s,n]] @ Wn^T) )
 with T_b[v] = pos_table[v] + (h[b][v-1] if v>=1 else 0)   (v in [0, 2049))
Since the mask multiplies matmul outputs and matmul is linear:
 out[b,s,:] = ( sum_n T'_b[idx_eff[b,s,n]] ) @ (Wn^T/N)
 where T' has zero rows at 2049..2175 and idx_eff = mask ? idx : sink,
 with the sink SPREAD over the zero rows (2049+2*(col%64)) so masked
 gathers don't serialize on one HBM/SBUF row.

Sharding: data-parallel over batch, one NeuronCore per batch row (B == 8).

Per-core plan (v3):
 - T' built in SBUF bf16 then copied to a DRAM scratch table [2176, 128].
 - dma_gather HBM-source NON-transpose (4 SWDGE queues, ~2.4 ns/idx).
   The transposed (XBAR) gather corrupts data non-deterministically at
   full rate (verified on HW); the plain row-per-partition path is clean.
 - gather call c covers s-chunk U=c//8, n in [4*(c%8), +4); position
   i = 128*a + k -> gblk[k, block, :] = T'[idx_eff[128U + k, 4*(c%8)+a]]
   i.e. each 128-row block is one (U, n) pair with s along partitions.
 - n-sum on PE: psum[s,h] += I^T @ block  (32 identity-matmuls/chunk).
 - per chunk: m -> bf16 -> PE transpose -> mT; out[s,k] = mT^T @ wnt.
 - All mid-stream copies on the Activation engine: concurrent DVE ops
   can corrupt the gather ucode's index stream (shared POOL/DVE port).
"""
import sys

sys.path.insert(0, '/opt/trn_rl_repo')

import numpy as np

import concourse.bacc as bacc
import concourse.bass as bass
import concourse.mybir as mybir
import concourse.tile as tile
from concourse.bass_utils import run_bass_kernel_spmd
from concourse.masks import make_identity

B, N, H = 8, 32, 128
NI = 512             # idxs per dma_gather call (ucode ring ceiling)
UBLK = 512           # s rows per pipeline block
F32 = mybir.dt.float32
I32 = mybir.dt.int32
I16 = mybir.dt.int16
BF16 = mybir.dt.bfloat16


def build_program(S: int = 2048, mode: str = "full"):
    VPOS = S + 1                      # pos_table rows; sinks start at VPOS
    NRANKS = (VPOS + 1 + 127) // 128  # table 128-row chunks, zero padded
    VPAD = NRANKS * 128
    nblk = S // UBLK if S >= UBLK else 1
    ublk = min(UBLK, S)
    calls = ublk * N // NI            # gather calls per pipeline block
    chunks = ublk // 128              # 128-s chunks per pipeline block
    acols = S * N // 16               # wrapped idx cols, whole batch row

    nc = bacc.Bacc("TRN2", debug=False, num_swdge_queues=4)
    h_d = nc.dram_tensor("h", [S, H], F32, kind="ExternalInput")
    idx_d = nc.dram_tensor("idx", [S, N], I32, kind="ExternalInput")
    msk_d = nc.dram_tensor("msk", [S, N], I32, kind="ExternalInput")
    pos_d = nc.dram_tensor("pos", [VPOS, H], F32, kind="ExternalInput")
    wn_d = nc.dram_tensor("wn", [H, H], F32, kind="ExternalInput")
    out_d = nc.dram_tensor("out", [S, H], F32, kind="ExternalOutput")
    tbl_d = nc.dram_tensor("tscratch", [VPAD, H], BF16, kind="Internal")
    dump_d = None
    if mode == "nomm":
        dump_d = nc.dram_tensor("gdump", [128, S * N], mybir.dt.uint16,
                                kind="ExternalOutput")

    with tile.TileContext(nc) as tc:
        with (
            tc.tile_pool(name="const", bufs=1) as constp,
            tc.tile_pool(name="stage", bufs=3) as stagep,
            tc.tile_pool(name="prep", bufs=1) as prepp,
            tc.tile_pool(name="gblk", bufs=2) as gblkp,
            tc.tile_pool(name="outp", bufs=4) as outp,
            tc.tile_pool(name="psA", bufs=2, space="PSUM") as psA,
            tc.tile_pool(name="psB", bufs=2, space="PSUM") as psB,
            tc.tile_pool(name="psC", bufs=2, space="PSUM") as psC,
        ):
            # ---- raw wrapped indices: longest DMA first ---------------
            # natural layout: idxn[p, 32k + n] = idx[16k + p, n]
            # (per partition 128 contiguous 128B runs).
            idxn = prepp.tile([16, acols], I32, tag="idxn")
            mskn = prepp.tile([16, acols], I32, tag="mskn")
            nc.sync.dma_start(
                idxn[:].rearrange("p (k n) -> p k n", n=N),
                idx_d[:].rearrange("(k p) n -> p k n", p=16))
            nc.scalar.dma_start(
                mskn[:].rearrange("p (k n) -> p k n", n=N),
                msk_d[:].rearrange("(k p) n -> p k n", p=16))

            # sink pattern early: the Pool engine is idle here.
            # masked -> spread sink rows 2049+2*(col%64) (zero table rows);
            # a single sink row would serialize gather reads on one partition.
            idxe = prepp.tile([16, acols], I32, tag="idxe")
            nc.gpsimd.iota(
                idxe[:].rearrange("p (r c) -> p r c", c=64),
                pattern=[[0, acols // 64], [2, 64]],
                base=VPOS,
                channel_multiplier=0,
            )

            # ---- Wn^T / N in bf16 ------------------------------------
            wn_sb = constp.tile([H, H], F32)
            nc.sync.dma_start(wn_sb[:], wn_d[:])
            ident = constp.tile([128, 128], F32)
            make_identity(nc, ident[:])
            identb = constp.tile([128, 128], BF16)
            nc.vector.tensor_copy(identb[:], ident[:])
            wnt_ps = psC.tile([128, H], F32, tag="wntps")
            nc.tensor.transpose(out=wnt_ps[:], in_=wn_sb[:], identity=ident[:])
            wnt = constp.tile([H, H], BF16)
            nc.vector.tensor_scalar_mul(wnt[:], wnt_ps[:], 1.0 / N)

            # ---- fused table T' -> SBUF bf16 -> DRAM scratch ---------
            # tbl[p, q*H:(q+1)*H] = T'[q*128 + p, :]; rows VPOS..VPAD-1 = 0
            tbl = constp.tile([128, NRANKS * H], BF16)
            for q in range(NRANKS):
                v0 = q * 128
                n_pos = min(128, VPOS - v0)
                if n_pos <= 0:
                    nc.gpsimd.memset(tbl[:, q * H:(q + 1) * H], 0.0)
                    continue
                pstage = stagep.tile([128, H], F32, tag="pstage")
                hstage = stagep.tile([128, H], F32, tag="hstage")
                if n_pos < 128:
                    nc.gpsimd.memset(tbl[:, q * H:(q + 1) * H], 0.0)
                nc.sync.dma_start(pstage[:n_pos, :], pos_d[v0:v0 + n_pos, :])
                if q == 0:
                    nc.gpsimd.memset(hstage[0:1, :], 0.0)
                    nc.sync.dma_start(hstage[1:n_pos, :], h_d[0:n_pos - 1, :])
                else:
                    nc.scalar.dma_start(
                        hstage[:n_pos, :], h_d[v0 - 1:v0 + n_pos - 1, :])
                nc.vector.tensor_add(
                    tbl[:n_pos, q * H:(q + 1) * H], pstage[:n_pos, :], hstage[:n_pos, :]
                )
            # write chunks to the DRAM scratch as they complete
            tdv = tbl_d[:].rearrange("(q p) e -> p q e", p=128)
            for q in range(NRANKS):
                eng = nc.sync if q % 2 == 0 else nc.scalar
                eng.dma_start(
                    tdv[:, q:q + 1, :],
                    tbl[:, q * H:(q + 1) * H].rearrange(
                        "p (one e) -> p one e", one=1))

            # ---- mask fold + fused narrow+permute (prologue only) ----
            nc.vector.copy_predicated(idxe[:], mskn[:], idxn[:])
            # fused i32->i16 narrow + permute natural (kk nh a) -> gather
            # order (nh a kk) per U, split across DVE and Act engines:
            # idxbuf[p, 256U + 32nh + 8a + kk] = idx_eff[128U + 16kk + p,
            #                                            4nh + a]
            idxbuf = prepp.tile([128, acols], I16, tag="idxbuf")
            idxe16 = idxe[:].bitcast(I16)
            for u in range(S // 128):
                src = idxe16[:, 512 * u:512 * (u + 1)].rearrange(
                    "p (kk nh a two) -> p nh a kk two", nh=8, a=4, two=2)
                dst = idxbuf[0:16, 256 * u:256 * (u + 1)].rearrange(
                    "p (nh a kk one) -> p nh a kk one", a=4, kk=8, one=1)
                eng = nc.vector if u % 2 == 0 else nc.scalar
                if u % 2 == 0:
                    nc.vector.tensor_copy(dst, src[:, :, :, :, 0:1])
                else:
                    nc.scalar.copy(dst, src[:, :, :, :, 0:1])
            # replicate the final int16 to the other 7 16-partition groups
            for r in range(1, 8):
                eng = nc.sync if r % 2 == 0 else nc.scalar
                eng.dma_start(idxbuf[16 * r:16 * (r + 1), :], idxbuf[0:16, :])

            for bi in range(nblk):
                s0 = bi * ublk
                # ---- gathers: call c -> 4 blocks of (U, n) rows ------
                gblk = gblkp.tile([128, chunks * N, H], BF16, tag="gblk")
                for c in range(calls):
                    cg = bi * calls + c         # global call number
                    wc0 = cg * (NI // 16)
                    nc.gpsimd.dma_gather(
                        gblk[:, 4 * c:4 * c + 4, :],
                        tbl_d[:],
                        idxbuf[:, wc0:wc0 + NI // 16],
                        NI, NI, H,
                        transpose=False,
                        queue_num=cg % 4,
                    )

                if mode == "nomm":
                    nc.scalar.dma_start(
                        dump_d[:].rearrange(
                            "p (x e) -> p x e", e=H)[:, s0 * N // 128:
                                                     (s0 + ublk) * N // 128, :],
                        gblk[:].bitcast(mybir.dt.uint16))
                    continue

                # ---- per 128-s chunk: n-sum, transpose, Wn -----------
                for uu in range(chunks):
                    psm = psA.tile([128, H], F32, tag="psm")
                    for n in range(N):
                        nc.tensor.matmul(
                            out=psm[:],
                            lhsT=identb[:],
                            rhs=gblk[:, N * uu + n, :],
                            start=(n == 0),
                            stop=(n == N - 1),
                        )
                    msb = outp.tile([128, H], BF16, tag="msb")
                    nc.scalar.copy(msb[:], psm[:])
                    pst = psB.tile([128, H], BF16, tag="pst")
                    nc.tensor.transpose(
                        out=pst[:], in_=msb[:], identity=identb[:])
                    mT = outp.tile([128, H], BF16, tag="mT")
                    nc.scalar.copy(mT[:], pst[:])
                    pso = psC.tile([128, H], F32, tag="pso")
                    nc.tensor.matmul(
                        out=pso[:], lhsT=mT[:], rhs=wnt[:],
                        start=True, stop=True)
                    osb = outp.tile([128, H], F32, tag="osb")
                    nc.scalar.copy(osb[:], pso[:])
                    eng = nc.sync if uu % 2 == 0 else nc.scalar
                    eng.dma_start(
                        out_d[s0 + uu * 128:s0 + (uu + 1) * 128, :], osb[:]
                    )

    nc.compile()
    return nc


_CACHE: dict[int, object] = {}


def _get_program(S: int):
    if S not in _CACHE:
        _CACHE[S] = build_program(S)
    return _CACHE[S]


def kernel(x, h, g, neighbor_index, neighbor_mask, pos_table, Wn):
    """Full inputs in, full output out. x and g are unused by the math
    (g only provides the zero row shape; x is unused in the reference)."""
    h = np.asarray(h)
    idx = np.asarray(neighbor_index)
    msk = np.asarray(neighbor_mask)
    pos = np.ascontiguousarray(np.asarray(pos_table), dtype=np.float32)
    wn = np.ascontiguousarray(np.asarray(Wn), dtype=np.float32)
    b, s, n = idx.shape
    assert (b, n) == (B, N) and h.shape == (B, s, H)

    nc = _get_program(s)
    in_maps = [
        {
            "h": np.ascontiguousarray(h[c], dtype=np.float32),
            "idx": np.ascontiguousarray(idx[c], dtype=np.int32),
            "msk": np.ascontiguousarray(msk[c], dtype=np.int32),
            "pos": pos,
            "wn": wn,
        }
        for c in range(B)
    ]
    res = run_bass_kernel_spmd(nc, in_maps, core_ids=list(range(B)))
    return np.stack([res.results[c]["out"] for c in range(B)], axis=0)


# revision 22
# speedup vs baseline: 1.0193x; 1.0193x over previous
"""Trainium2 Bass kernel for nn_Neighbor_Mean (gnn message passing).

Math: out[b,s,:] = mean_n( mask[b,s,n] * (T_b[idx[b,s,n]] @ Wn^T) )
 with T_b[v] = pos_table[v] + (h[b][v-1] if v>=1 else 0)   (v in [0, 2049))
Since the mask multiplies matmul outputs and matmul is linear:
 out[b,s,:] = ( sum_n T'_b[idx_eff[b,s,n]] ) @ (Wn^T/N)
 where T' has zero rows at 2049..2175 and idx_eff = mask ? idx : sink,
 with the sink SPREAD over the zero rows (2049+2*(col%64)) so masked
 gathers don't serialize on one HBM/SBUF row.

Sharding: data-parallel over batch, one NeuronCore per batch row (B == 8).

Per-core plan (v3):
 - T' built in SBUF bf16 then copied to a DRAM scratch table [2176, 128].
 - dma_gather HBM-source NON-transpose (4 SWDGE queues, ~2.4 ns/idx).
   The transposed (XBAR) gather corrupts data non-deterministically at
   full rate (verified on HW); the plain row-per-partition path is clean.
 - gather call c covers s-chunk U=c//8, n in [4*(c%8), +4); position
   i = 128*a + k -> gblk[k, block, :] = T'[idx_eff[128U + k, 4*(c%8)+a]]
   i.e. each 128-row block is one (U, n) pair with s along partitions.
 - n-sum on PE: psum[s,h] += I^T @ block  (32 identity-matmuls/chunk).
 - per chunk: m -> bf16 -> PE transpose -> mT; out[s,k] = mT^T @ wnt.
 - All mid-stream copies on the Activation engine: concurrent DVE ops
   can corrupt the gather ucode's index stream (shared POOL/DVE port).
"""
import sys

sys.path.insert(0, '/opt/trn_rl_repo')

import numpy as np

import concourse.bacc as bacc
import concourse.bass as bass
import concourse.mybir as mybir
import concourse.tile as tile
from concourse.bass_utils import run_bass_kernel_spmd
from concourse.masks import make_identity

B, N, H = 8, 32, 128
NI = 512             # idxs per dma_gather call (ucode ring ceiling)
UBLK = 512           # s rows per pipeline block
F32 = mybir.dt.float32
I32 = mybir.dt.int32
I16 = mybir.dt.int16
BF16 = mybir.dt.bfloat16


def build_program(S: int = 2048, mode: str = "full"):
    VPOS = S + 1                      # pos_table rows; sinks start at VPOS
    NRANKS = (VPOS + 1 + 127) // 128  # table 128-row chunks, zero padded
    VPAD = NRANKS * 128
    nblk = S // UBLK if S >= UBLK else 1
    ublk = min(UBLK, S)
    calls = ublk * N // NI            # gather calls per pipeline block
    chunks = ublk // 128              # 128-s chunks per pipeline block
    acols = S * N // 16               # wrapped idx cols, whole batch row

    nc = bacc.Bacc("TRN2", debug=False, num_swdge_queues=4)
    h_d = nc.dram_tensor("h", [S, H], F32, kind="ExternalInput")
    idx_d = nc.dram_tensor("idx", [S, N], I32, kind="ExternalInput")
    msk_d = nc.dram_tensor("msk", [S, N], I32, kind="ExternalInput")
    pos_d = nc.dram_tensor("pos", [VPOS, H], F32, kind="ExternalInput")
    wn_d = nc.dram_tensor("wn", [H, H], F32, kind="ExternalInput")
    out_d = nc.dram_tensor("out", [S, H], F32, kind="ExternalOutput")
    tbl_d = nc.dram_tensor("tscratch", [VPAD, H], BF16, kind="Internal")
    dump_d = None
    if mode == "nomm":
        dump_d = nc.dram_tensor("gdump", [128, S * N], mybir.dt.uint16,
                                kind="ExternalOutput")

    with tile.TileContext(nc) as tc:
        with (
            tc.tile_pool(name="const", bufs=1) as constp,
            tc.tile_pool(name="stage", bufs=3) as stagep,
            tc.tile_pool(name="prep", bufs=1) as prepp,
            tc.tile_pool(name="gblk", bufs=2) as gblkp,
            tc.tile_pool(name="outp", bufs=4) as outp,
            tc.tile_pool(name="psA", bufs=2, space="PSUM") as psA,
            tc.tile_pool(name="psB", bufs=2, space="PSUM") as psB,
            tc.tile_pool(name="psC", bufs=2, space="PSUM") as psC,
        ):
            # ---- raw wrapped indices: longest DMA first ---------------
            # natural layout: idxn[p, 32k + n] = idx[16k + p, n]
            # (per partition 128 contiguous 128B runs).
            idxn = prepp.tile([16, acols], I32, tag="idxn")
            mskn = prepp.tile([16, acols], I32, tag="mskn")
            nc.sync.dma_start(
                idxn[:].rearrange("p (k n) -> p k n", n=N),
                idx_d[:].rearrange("(k p) n -> p k n", p=16))
            nc.scalar.dma_start(
                mskn[:].rearrange("p (k n) -> p k n", n=N),
                msk_d[:].rearrange("(k p) n -> p k n", p=16))

            # sink pattern early: the Pool engine is idle here.
            # masked -> spread sink rows 2049+2*(col%64) (zero table rows);
            # a single sink row would serialize gather reads on one partition.
            idxe = prepp.tile([16, acols], I32, tag="idxe")
            nc.gpsimd.iota(
                idxe[:].rearrange("p (r c) -> p r c", c=64),
                pattern=[[0, acols // 64], [2, 64]],
                base=VPOS,
                channel_multiplier=0,
            )

            # ---- Wn^T / N in bf16 ------------------------------------
            wn_sb = constp.tile([H, H], F32)
            nc.sync.dma_start(wn_sb[:], wn_d[:])
            ident = constp.tile([128, 128], F32)
            make_identity(nc, ident[:])
            identb = constp.tile([128, 128], BF16)
            nc.vector.tensor_copy(identb[:], ident[:])
            wnt_ps = psC.tile([128, H], F32, tag="wntps")
            nc.tensor.transpose(out=wnt_ps[:], in_=wn_sb[:], identity=ident[:])
            wnt = constp.tile([H, H], BF16)
            nc.vector.tensor_scalar_mul(wnt[:], wnt_ps[:], 1.0 / N)

            # ---- fused table T' -> SBUF bf16 -> DRAM scratch ---------
            # tbl[p, q*H:(q+1)*H] = T'[q*128 + p, :]; rows VPOS..VPAD-1 = 0
            tbl = constp.tile([128, NRANKS * H], BF16)
            for q in range(NRANKS):
                v0 = q * 128
                n_pos = min(128, VPOS - v0)
                if n_pos <= 0:
                    nc.gpsimd.memset(tbl[:, q * H:(q + 1) * H], 0.0)
                    continue
                pstage = stagep.tile([128, H], F32, tag="pstage")
                hstage = stagep.tile([128, H], F32, tag="hstage")
                if n_pos < 128:
                    nc.gpsimd.memset(tbl[:, q * H:(q + 1) * H], 0.0)
                nc.sync.dma_start(pstage[:n_pos, :], pos_d[v0:v0 + n_pos, :])
                if q == 0:
                    nc.gpsimd.memset(hstage[0:1, :], 0.0)
                    nc.sync.dma_start(hstage[1:n_pos, :], h_d[0:n_pos - 1, :])
                else:
                    nc.scalar.dma_start(
                        hstage[:n_pos, :], h_d[v0 - 1:v0 + n_pos - 1, :])
                nc.vector.tensor_add(
                    tbl[:n_pos, q * H:(q + 1) * H], pstage[:n_pos, :], hstage[:n_pos, :]
                )
            # write chunks to the DRAM scratch as they complete
            tdv = tbl_d[:].rearrange("(q p) e -> p q e", p=128)
            for q in range(NRANKS):
                eng = nc.sync if q % 2 == 0 else nc.scalar
                eng.dma_start(
                    tdv[:, q:q + 1, :],
                    tbl[:, q * H:(q + 1) * H].rearrange(
                        "p (one e) -> p one e", one=1))

            # ---- mask fold + fused narrow+permute (prologue only) ----
            nc.vector.copy_predicated(idxe[:], mskn[:], idxn[:])
            # fused i32->i16 narrow + permute natural (kk nh a) -> gather
            # order (nh a kk) per U, split across DVE and Act engines:
            # idxbuf[p, 256U + 32nh + 8a + kk] = idx_eff[128U + 16kk + p,
            #                                            4nh + a]
            idxbuf = prepp.tile([128, acols], I16, tag="idxbuf")
            idxe16 = idxe[:].bitcast(I16)
            for u in range(S // 128):
                src = idxe16[:, 512 * u:512 * (u + 1)].rearrange(
                    "p (kk nh a two) -> p nh a kk two", nh=8, a=4, two=2)
                dst = idxbuf[0:16, 256 * u:256 * (u + 1)].rearrange(
                    "p (nh a kk one) -> p nh a kk one", a=4, kk=8, one=1)
                nc.vector.tensor_copy(dst, src[:, :, :, :, 0:1])
            # replicate the final int16 to the other 7 16-partition groups
            for r in range(1, 8):
                eng = nc.sync if r % 2 == 0 else nc.scalar
                eng.dma_start(idxbuf[16 * r:16 * (r + 1), :], idxbuf[0:16, :])

            for bi in range(nblk):
                s0 = bi * ublk
                # ---- gathers: call c -> 4 blocks of (U, n) rows ------
                gblk = gblkp.tile([128, chunks * N, H], BF16, tag="gblk")
                for c in range(calls):
                    cg = bi * calls + c         # global call number
                    wc0 = cg * (NI // 16)
                    nc.gpsimd.dma_gather(
                        gblk[:, 4 * c:4 * c + 4, :],
                        tbl_d[:],
                        idxbuf[:, wc0:wc0 + NI // 16],
                        NI, NI, H,
                        transpose=False,
                        queue_num=cg % 4,
                    )

                if mode == "nomm":
                    nc.scalar.dma_start(
                        dump_d[:].rearrange(
                            "p (x e) -> p x e", e=H)[:, s0 * N // 128:
                                                     (s0 + ublk) * N // 128, :],
                        gblk[:].bitcast(mybir.dt.uint16))
                    continue

                # ---- per 128-s chunk: n-sum, transpose, Wn -----------
                for uu in range(chunks):
                    psm = psA.tile([128, H], F32, tag="psm")
                    for n in range(N):
                        nc.tensor.matmul(
                            out=psm[:],
                            lhsT=identb[:],
                            rhs=gblk[:, N * uu + n, :],
                            start=(n == 0),
                            stop=(n == N - 1),
                        )
                    msb = outp.tile([128, H], BF16, tag="msb")
                    nc.scalar.copy(msb[:], psm[:])
                    pst = psB.tile([128, H], BF16, tag="pst")
                    nc.tensor.transpose(
                        out=pst[:], in_=msb[:], identity=identb[:])
                    mT = outp.tile([128, H], BF16, tag="mT")
                    nc.scalar.copy(mT[:], pst[:])
                    pso = psC.tile([128, H], F32, tag="pso")
                    nc.tensor.matmul(
                        out=pso[:], lhsT=mT[:], rhs=wnt[:],
                        start=True, stop=True)
                    osb = outp.tile([128, H], F32, tag="osb")
                    nc.scalar.copy(osb[:], pso[:])
                    eng = nc.sync if uu % 2 == 0 else nc.scalar
                    eng.dma_start(
                        out_d[s0 + uu * 128:s0 + (uu + 1) * 128, :], osb[:]
                    )

    nc.compile()
    return nc


_CACHE: dict[int, object] = {}


def _get_program(S: int):
    if S not in _CACHE:
        _CACHE[S] = build_program(S)
    return _CACHE[S]


def kernel(x, h, g, neighbor_index, neighbor_mask, pos_table, Wn):
    """Full inputs in, full output out. x and g are unused by the math
    (g only provides the zero row shape; x is unused in the reference)."""
    h = np.asarray(h)
    idx = np.asarray(neighbor_index)
    msk = np.asarray(neighbor_mask)
    pos = np.ascontiguousarray(np.asarray(pos_table), dtype=np.float32)
    wn = np.ascontiguousarray(np.asarray(Wn), dtype=np.float32)
    b, s, n = idx.shape
    assert (b, n) == (B, N) and h.shape == (B, s, H)

    nc = _get_program(s)
    in_maps = [
        {
            "h": np.ascontiguousarray(h[c], dtype=np.float32),
            "idx": np.ascontiguousarray(idx[c], dtype=np.int32),
            "msk": np.ascontiguousarray(msk[c], dtype=np.int32),
            "pos": pos,
            "wn": wn,
        }
        for c in range(B)
    ]
    res = run_bass_kernel_spmd(nc, in_maps, core_ids=list(range(B)))
    return np.stack([res.results[c]["out"] for c in range(B)], axis=0)
